# revision 5
# baseline (speedup 1.0000x reference)
"""Chamfer distance loss kernel for 8 Trainium2 NeuronCores.

Problem: template/source point clouds [B=4, N=8192, 3] fp32.
  d2[b,n,m] = ||t[b,n] - s[b,m]||^2
  out = mean_b( (mean_n sqrt(min_m d2) + mean_m sqrt(min_n d2)) / 2 )

Sharding: core c handles batch b=c//2, template-half h=c%2 (4096 rows of
the [8192, 8192] distance matrix).  Unlike a two-pass design, each d2
entry is computed ONCE and serves both reduction directions:

  per 128-row strip (16 matmuls of [128, 512] into all 8 PSUM banks):
    PE : d2 = a2[n] + b2[m] - 2 t.s via one K=16 f32r contraction
         (hi/lo split of operands reproduces fp32 products; a2/b2 are
          folded in as extra contraction rows against ones)
    ACT: transcodes PSUM fp32 -> SBUF fp16 (two [128, 4096] copies)
    DVE: row path  - one tensor_tensor_reduce fusing the half-vs-half
         elementwise min with a min-reduce -> rowmin[strip] ([128, 1])
         col path  - one fp16 tensor_tensor min (2x_1p mode) folding the
         strip into a running [128, 8192] column accumulator

DVE is the bottleneck engine (~8.3K cycles/strip vs 17.3K for the
reduce-only baseline); ACT (~8.5K cycles at 1.2 GHz) and PE (~3.5us)
hide underneath it.  The kernel ships the raw col accumulator and
rowmins; host does the cross-partition / cross-core mins, sqrt and the
means (tiny: 8x(2MB+16KB) gathered once).
"""

import numpy as np

B = 4
N = 8192  # points per cloud
HALF = N // 2  # template rows per core
N_CORES = 8
STRIPS = HALF // 128  # 32
K_ROWS = 16  # hi/lo-split contraction rows (12 products + a2h/l + b2h/l)

_cache = {}


def _build_bass(reps=1):
    import contextlib
    from concourse import bacc, mybir, tile

    f32 = mybir.dt.float32
    f16 = mybir.dt.float16
    f32r = mybir.dt.float32r
    AOp = mybir.AluOpType

    nc = bacc.Bacc("TRN2", target_bir_lowering=False, debug=False,
                   num_devices=N_CORES)

    lhs = nc.dram_tensor("lhs", [K_ROWS, HALF], f32r,
                         kind="ExternalInput").ap()
    rhs = nc.dram_tensor("rhs", [K_ROWS, N], f32r,
                         kind="ExternalInput").ap()
    out_acc = nc.dram_tensor("out_acc", [128, N], f16,
                             kind="ExternalOutput").ap()
    out_row = nc.dram_tensor("out_row", [128, STRIPS], f32,
                             kind="ExternalOutput").ap()

    with tile.TileContext(nc) as tc:
        with tc.tile_pool(name="const", bufs=1) as cpool, \
             tc.tile_pool(name="psum", bufs=1, space="PSUM") as ppool, \
             tc.tile_pool(name="strips", bufs=2) as spool, \
             tc.tile_pool(name="scratch", bufs=1) as qpool:

            lhs_sb = cpool.tile([K_ROWS, HALF], f32r, tag="lhs")
            rhs_sb = cpool.tile([K_ROWS, N], f32r, tag="rhs")
            acc = cpool.tile([128, N], f16, tag="acc")
            rowmins = cpool.tile([128, STRIPS], f32, tag="rowmins")
            # row-path fold tree scratch (fp16 keeps DVE in 2x_1p mode)
            f1 = qpool.tile([128, 4096], f16, tag="f1")
            f2 = qpool.tile([128, 2048], f16, tag="f2")
            f3 = qpool.tile([128, 1024], f16, tag="f3")
            f4 = qpool.tile([128, 512], f16, tag="f4")

            nc.sync.dma_start(lhs_sb[:, :], lhs)
            nc.sync.dma_start(rhs_sb[:, :], rhs)

            P = ppool.tile([128, 8 * 512], f32, tag="P")

            loop_ctx = (tc.For_i(0, reps, 1) if reps > 1
                        else contextlib.nullcontext())
            with loop_ctx:
                for s in range(STRIPS):
                    # strip 0 lands directly in the accumulator (saves one
                    # col fold and doubles as per-rep re-initialization)
                    dst = acc if s == 0 else spool.tile([128, N], f16,
                                                        tag="strip")
                    for g in range(4):
                        base = 2048 * (g % 2)
                        for j in range(4):
                            m = 4 * g + j
                            nc.tensor.matmul(
                                P[:, base + 512 * j: base + 512 * (j + 1)],
                                lhsT=lhs_sb[:, 128 * s: 128 * (s + 1)],
                                rhs=rhs_sb[:, 512 * m: 512 * (m + 1)],
                                start=True, stop=True,
                            )
                        nc.scalar.copy(dst[:, 2048 * g: 2048 * (g + 1)],
                                       P[:, base: base + 2048])
                    # row path: binary fold tree at 2x, then one 1x reduce
                    nc.vector.tensor_tensor(
                        out=f1[:, :], in0=dst[:, 0:4096], in1=dst[:, 4096:N],
                        op=AOp.min)
                    nc.vector.tensor_tensor(
                        out=f2[:, :], in0=f1[:, 0:2048], in1=f1[:, 2048:4096],
                        op=AOp.min)
                    nc.vector.tensor_tensor(
                        out=f3[:, :], in0=f2[:, 0:1024], in1=f2[:, 1024:2048],
                        op=AOp.min)
                    nc.vector.tensor_tensor(
                        out=f4[:, :], in0=f3[:, 0:512], in1=f3[:, 512:1024],
                        op=AOp.min)
                    nc.vector.tensor_reduce(
                        rowmins[:, s:s + 1], f4[:, :],
                        axis=mybir.AxisListType.X, op=AOp.min)
                    if s > 0:
                        nc.vector.tensor_tensor(
                            out=acc[:, :], in0=acc[:, :], in1=dst[:, :],
                            op=AOp.min,
                        )

                nc.sync.dma_start(out_acc, acc[:, :])
                nc.sync.dma_start(out_row, rowmins[:, :])

    nc.compile()
    return nc


def _rnd11(x):
    """Round-to-nearest keeping 11 explicit mantissa bits (the rounding the
    PE applies to float32r operands, measured on HW)."""
    xi = x.view(np.uint32).astype(np.uint64)
    out = ((xi + np.uint64(1 << 11)) & np.uint64(0xFFFFF000)).astype(np.uint32)
    return out.view(np.float32)


def _hilo(x):
    hi = _rnd11(np.ascontiguousarray(x, np.float32))
    lo = _rnd11((x - hi).astype(np.float32))
    return hi, lo


def _prep_core_inputs(template, source, c):
    b, h = divmod(c, 2)
    tch = template[b, h * HALF:(h + 1) * HALF]  # [4096, 3] rows
    sfull = source[b]  # [8192, 3] cols

    def sq(x):  # |x|^2 per point, fp32
        return (x * x).sum(axis=-1, dtype=np.float32)

    # Contraction layout (k: lhs row x rhs row):
    #   0-2 : -2*t_hi . s_hi     3-5 : -2*t_hi . s_lo
    #   6-8 : -2*t_lo . s_hi     9-11: -2*t_lo . s_lo
    #   12  : a2_hi * 1          13  : a2_lo * 1
    #   14  : 1 * b2_hi          15  : 1 * b2_lo
    v = (-2.0 * tch.T).astype(np.float32)  # [3, 4096]
    vh, vl = _hilo(v)
    a2h, a2l = _hilo(sq(tch)[None])
    ones_n = np.ones((1, tch.shape[0]), np.float32)
    lhs = np.concatenate([vh, vh, vl, vl, a2h, a2l, ones_n, ones_n], axis=0)

    w = np.ascontiguousarray(sfull.T, np.float32)  # [3, 8192]
    wh, wl = _hilo(w)
    b2h, b2l = _hilo(sq(sfull)[None])
    ones_m = np.ones((1, N), np.float32)
    rhs = np.concatenate([wh, wl, wh, wl, ones_m, ones_m, b2h, b2l], axis=0)

    return {
        "lhs": np.ascontiguousarray(lhs),
        "rhs": np.ascontiguousarray(rhs),
    }


def _run(template, source, trace=False):
    from concourse.bass_utils import run_bass_kernel_spmd

    template = np.asarray(template, np.float32)
    source = np.asarray(source, np.float32)
    assert template.shape == (B, N, 3) and source.shape == (B, N, 3)

    if "nc" not in _cache:
        _cache["nc"] = _build_bass()
    nc = _cache["nc"]

    in_maps = [_prep_core_inputs(template, source, c) for c in range(N_CORES)]
    res = run_bass_kernel_spmd(nc, in_maps, core_ids=list(range(N_CORES)),
                               trace=trace)

    acc = np.stack([np.asarray(r["out_acc"], np.float32)
                    for r in res.results])  # [8, 128, 8192]
    rm = np.stack([np.asarray(r["out_row"], np.float32)
                   for r in res.results])  # [8, 128, 32]

    # template -> source: rowmins are complete minima for this core's rows
    rowsqrt = np.sqrt(np.maximum(rm, 0.0)).sum(axis=(1, 2))  # [8]
    cost01 = (rowsqrt[0::2] + rowsqrt[1::2]) / N  # [4]

    # source -> template: min over partitions, then across the core pair
    colmin_core = acc.min(axis=1)  # [8, 8192]
    colmin = np.minimum(colmin_core[0::2], colmin_core[1::2])  # [4, 8192]
    cost10 = np.sqrt(np.maximum(colmin, 0.0)).mean(axis=1)  # [4]

    chamfer = ((cost01 + cost10) / 2.0).mean()
    return np.asarray(chamfer, dtype=np.float32), res


def kernel(template, source):
    val, _ = _run(template, source, trace=False)
    return val


# revision 25
# speedup vs baseline: 1.0896x; 1.0896x over previous
"""Chamfer distance loss kernel for 8 Trainium2 NeuronCores.

Problem: template/source point clouds [B=4, N=8192, 3] fp32.
  d2[b,n,m] = ||t[b,n] - s[b,m]||^2
  out = mean_b( (mean_n sqrt(min_m d2) + mean_m sqrt(min_n d2)) / 2 )

Sharding: core c handles batch b=c//2, template-half h=c%2 (4096 rows of
the [8192, 8192] distance matrix).  Unlike a two-pass design, each d2
entry is computed ONCE and serves both reduction directions:

  per 128-row strip (16 matmuls of [128, 512] into all 8 PSUM banks):
    PE : d2 = a2[n] + b2[m] - 2 t.s via one K=16 f32r contraction
         (hi/lo split of operands reproduces fp32 products; a2/b2 are
          folded in as extra contraction rows against ones)
    ACT: transcodes PSUM fp32 -> SBUF fp16 (two [128, 4096] copies)
    DVE: row path  - fp16 binary fold tree to 128 els (2x_1p mode,
         2 elem/cycle/lane) then a final 1x reduce -> rowmin[strip]
         col path  - one fp16 tensor_tensor min (2x_1p mode) folding the
         strip into a running [128, 8192] column accumulator

DVE is the bottleneck engine (~8.8K cycles/strip vs 17.3K for the
reduce-only fp32 baseline); ACT (~8.9K cycles at 1.2 GHz) and PE
(~3.5us) hide underneath it.  The kernel ships the raw col accumulator and
rowmins; host does the cross-partition / cross-core mins, sqrt and the
means (tiny: 8x(2MB+16KB) gathered once).
"""

import numpy as np

B = 4
N = 8192  # points per cloud
HALF = N // 2  # template rows per core
N_CORES = 8
STRIPS = HALF // 128  # 32
K_ROWS = 16  # hi/lo-split contraction rows (12 products + a2h/l + b2h/l)

_cache = {}


def _build_bass(reps=1):
    import contextlib
    from concourse import bacc, mybir, tile

    f32 = mybir.dt.float32
    f16 = mybir.dt.float16
    f32r = mybir.dt.float32r
    AOp = mybir.AluOpType

    nc = bacc.Bacc("TRN2", target_bir_lowering=False, debug=False,
                   num_devices=N_CORES)

    lhs = nc.dram_tensor("lhs", [K_ROWS, HALF], f32r,
                         kind="ExternalInput").ap()
    rhs = nc.dram_tensor("rhs", [K_ROWS, N], f32r,
                         kind="ExternalInput").ap()
    out_acc = nc.dram_tensor("out_acc", [128, N], f16,
                             kind="ExternalOutput").ap()
    out_row = nc.dram_tensor("out_row", [128, STRIPS], f32,
                             kind="ExternalOutput").ap()

    with tile.TileContext(nc) as tc:
        with tc.tile_pool(name="const", bufs=1) as cpool, \
             tc.tile_pool(name="psum", bufs=1, space="PSUM") as ppool, \
             tc.tile_pool(name="strips", bufs=2) as spool, \
             tc.tile_pool(name="scratch", bufs=1) as qpool:

            lhs_sb = cpool.tile([K_ROWS, HALF], f32r, tag="lhs")
            rhs_sb = cpool.tile([K_ROWS, N], f32r, tag="rhs")
            acc = cpool.tile([128, N], f16, tag="acc")
            rowmins = cpool.tile([128, STRIPS], f32, tag="rowmins")
            # row-path fold tree scratch (fp16 keeps DVE in 2x_1p mode)
            f1 = qpool.tile([128, 4096], f16, tag="f1")
            f2 = qpool.tile([128, 2048], f16, tag="f2")
            f3 = qpool.tile([128, 1024], f16, tag="f3")
            f4 = qpool.tile([128, 512], f16, tag="f4")
            # per-strip fold5 outputs land in 8-strip wave slots; one
            # segmented reduce per wave replaces 8 (fold6 + reduce) pairs
            f5buf = qpool.tile([128, 8, 256], f16, tag="f5buf")

            nc.sync.dma_start(lhs_sb[:, :], lhs)
            nc.sync.dma_start(rhs_sb[:, :], rhs)

            P = ppool.tile([128, 8 * 512], f32, tag="P")

            loop_ctx = (tc.For_i(0, reps, 1) if reps > 1
                        else contextlib.nullcontext())
            with loop_ctx:
                dst_prev = None
                for s in range(STRIPS):
                    dst = spool.tile([128, N], f16, tag="strip")
                    for g in range(4):
                        base = 2048 * (g % 2)
                        for j in range(4):
                            m = 4 * g + j
                            nc.tensor.matmul(
                                P[:, base + 512 * j: base + 512 * (j + 1)],
                                lhsT=lhs_sb[:, 128 * s: 128 * (s + 1)],
                                rhs=rhs_sb[:, 512 * m: 512 * (m + 1)],
                                start=True, stop=True,
                            )
                        nc.scalar.copy(dst[:, 2048 * g: 2048 * (g + 1)],
                                       P[:, base: base + 2048])
                    # row path: binary fold tree at 2x, then one 1x reduce
                    nc.vector.tensor_tensor(
                        out=f1[:, :], in0=dst[:, 0:4096], in1=dst[:, 4096:N],
                        op=AOp.min)
                    nc.vector.tensor_tensor(
                        out=f2[:, :], in0=f1[:, 0:2048], in1=f1[:, 2048:4096],
                        op=AOp.min)
                    nc.vector.tensor_tensor(
                        out=f3[:, :], in0=f2[:, 0:1024], in1=f2[:, 1024:2048],
                        op=AOp.min)
                    nc.vector.tensor_tensor(
                        out=f4[:, :], in0=f3[:, 0:512], in1=f3[:, 512:1024],
                        op=AOp.min)
                    nc.vector.tensor_tensor(
                        out=f5buf[:, s % 8, :], in0=f4[:, 0:256],
                        in1=f4[:, 256:512], op=AOp.min)
                    if s % 8 == 7:
                        nc.vector.tensor_reduce(
                            rowmins[:, s - 7:s + 1], f5buf[:, :, :],
                            axis=mybir.AxisListType.X, op=AOp.min)
                    # col path: acc initialized at s=1 from the first two
                    # strips (not at s=0) so the previous iteration's acc
                    # DMA-out overlaps ~12us of fresh compute before the
                    # first write-after-read on acc.
                    if s == 1:
                        nc.vector.tensor_tensor(
                            out=acc[:, :], in0=dst_prev[:, :], in1=dst[:, :],
                            op=AOp.min,
                        )
                    elif s > 1:
                        nc.vector.tensor_tensor(
                            out=acc[:, :], in0=acc[:, :], in1=dst[:, :],
                            op=AOp.min,
                        )
                    dst_prev = dst

                # split the 2MB result DMA across two DGE queues
                nc.sync.dma_start(out_acc[:, 0:N // 2], acc[:, 0:N // 2])
                nc.scalar.dma_start(out_acc[:, N // 2:N], acc[:, N // 2:N])
                nc.sync.dma_start(out_row, rowmins[:, :])

    nc.compile()
    return nc


def _rnd11(x):
    """Round-to-nearest keeping 11 explicit mantissa bits (the rounding the
    PE applies to float32r operands, measured on HW)."""
    xi = x.view(np.uint32).astype(np.uint64)
    out = ((xi + np.uint64(1 << 11)) & np.uint64(0xFFFFF000)).astype(np.uint32)
    return out.view(np.float32)


def _hilo(x):
    hi = _rnd11(np.ascontiguousarray(x, np.float32))
    lo = _rnd11((x - hi).astype(np.float32))
    return hi, lo


def _prep_core_inputs(template, source, c):
    b, h = divmod(c, 2)
    tch = template[b, h * HALF:(h + 1) * HALF]  # [4096, 3] rows
    sfull = source[b]  # [8192, 3] cols

    def sq(x):  # |x|^2 per point, fp32
        return (x * x).sum(axis=-1, dtype=np.float32)

    # Contraction layout (k: lhs row x rhs row):
    #   0-2 : -2*t_hi . s_hi     3-5 : -2*t_hi . s_lo
    #   6-8 : -2*t_lo . s_hi     9-11: -2*t_lo . s_lo
    #   12  : a2_hi * 1          13  : a2_lo * 1
    #   14  : 1 * b2_hi          15  : 1 * b2_lo
    v = (-2.0 * tch.T).astype(np.float32)  # [3, 4096]
    vh, vl = _hilo(v)
    a2h, a2l = _hilo(sq(tch)[None])
    ones_n = np.ones((1, tch.shape[0]), np.float32)
    lhs = np.concatenate([vh, vh, vl, vl, a2h, a2l, ones_n, ones_n], axis=0)

    w = np.ascontiguousarray(sfull.T, np.float32)  # [3, 8192]
    wh, wl = _hilo(w)
    b2h, b2l = _hilo(sq(sfull)[None])
    ones_m = np.ones((1, N), np.float32)
    rhs = np.concatenate([wh, wl, wh, wl, ones_m, ones_m, b2h, b2l], axis=0)

    return {
        "lhs": np.ascontiguousarray(lhs),
        "rhs": np.ascontiguousarray(rhs),
    }


def _run(template, source, trace=False):
    from concourse.bass_utils import run_bass_kernel_spmd

    template = np.asarray(template, np.float32)
    source = np.asarray(source, np.float32)
    assert template.shape == (B, N, 3) and source.shape == (B, N, 3)

    if "nc" not in _cache:
        _cache["nc"] = _build_bass()
    nc = _cache["nc"]

    in_maps = [_prep_core_inputs(template, source, c) for c in range(N_CORES)]
    res = run_bass_kernel_spmd(nc, in_maps, core_ids=list(range(N_CORES)),
                               trace=trace)

    acc = np.stack([np.asarray(r["out_acc"], np.float32)
                    for r in res.results])  # [8, 128, 8192]
    rm = np.stack([np.asarray(r["out_row"], np.float32)
                   for r in res.results])  # [8, 128, 32]

    # template -> source: rowmins are complete minima for this core's rows
    rowsqrt = np.sqrt(np.maximum(rm, 0.0)).sum(axis=(1, 2))  # [8]
    cost01 = (rowsqrt[0::2] + rowsqrt[1::2]) / N  # [4]

    # source -> template: min over partitions, then across the core pair
    colmin_core = acc.min(axis=1)  # [8, 8192]
    colmin = np.minimum(colmin_core[0::2], colmin_core[1::2])  # [4, 8192]
    cost10 = np.sqrt(np.maximum(colmin, 0.0)).mean(axis=1)  # [4]

    chamfer = ((cost01 + cost10) / 2.0).mean()
    return np.asarray(chamfer, dtype=np.float32), res


def kernel(template, source):
    val, _ = _run(template, source, trace=False)
    return val


# revision 26
# speedup vs baseline: 1.0938x; 1.0039x over previous
"""Chamfer distance loss kernel for 8 Trainium2 NeuronCores.

Problem: template/source point clouds [B=4, N=8192, 3] fp32.
  d2[b,n,m] = ||t[b,n] - s[b,m]||^2
  out = mean_b( (mean_n sqrt(min_m d2) + mean_m sqrt(min_n d2)) / 2 )

Sharding: core c handles batch b=c//2, template-half h=c%2 (4096 rows of
the [8192, 8192] distance matrix).  Unlike a two-pass design, each d2
entry is computed ONCE and serves both reduction directions:

  per 128-row strip (16 matmuls of [128, 512] into all 8 PSUM banks):
    PE : d2 = a2[n] + b2[m] - 2 t.s via one K=16 f32r contraction
         (hi/lo split of operands reproduces fp32 products; a2/b2 are
          folded in as extra contraction rows against ones)
    ACT: transcodes PSUM fp32 -> SBUF fp16 (two [128, 4096] copies)
    DVE: row path  - fp16 binary fold tree to 256 els (2x_1p mode,
         2 elem/cycle/lane) into per-strip wave slots; one segmented
         1x reduce per 8 strips -> rowmin[strip]
         col path  - one fp16 tensor_tensor min (2x_1p mode) folding the
         strip into a running [128, 8192] column accumulator

DVE is the bottleneck engine (~8.7K cycles/strip vs 17.3K for the
reduce-only fp32 baseline); ACT (~8.9K cycles at 1.2 GHz) and PE
(~3.5us) hide underneath it.  The kernel ships the raw col accumulator and
rowmins; host does the cross-partition / cross-core mins, sqrt and the
means (tiny: 8x(2MB+16KB) gathered once).
"""

import numpy as np

B = 4
N = 8192  # points per cloud
HALF = N // 2  # template rows per core
N_CORES = 8
STRIPS = HALF // 128  # 32
K_ROWS = 16  # hi/lo-split contraction rows (12 products + a2h/l + b2h/l)

_cache = {}


def _build_bass(reps=1):
    import contextlib
    from concourse import bacc, mybir, tile

    f32 = mybir.dt.float32
    f16 = mybir.dt.float16
    f32r = mybir.dt.float32r
    AOp = mybir.AluOpType

    nc = bacc.Bacc("TRN2", target_bir_lowering=False, debug=False,
                   num_devices=N_CORES)

    lhs = nc.dram_tensor("lhs", [K_ROWS, HALF], f32r,
                         kind="ExternalInput").ap()
    rhs = nc.dram_tensor("rhs", [K_ROWS, N], f32r,
                         kind="ExternalInput").ap()
    out_acc = nc.dram_tensor("out_acc", [128, N], f16,
                             kind="ExternalOutput").ap()
    out_row = nc.dram_tensor("out_row", [128, STRIPS], f32,
                             kind="ExternalOutput").ap()

    with tile.TileContext(nc) as tc:
        with tc.tile_pool(name="const", bufs=1) as cpool, \
             tc.tile_pool(name="psum", bufs=1, space="PSUM") as ppool, \
             tc.tile_pool(name="strips", bufs=2) as spool, \
             tc.tile_pool(name="scratch", bufs=1) as qpool:

            lhs_sb = cpool.tile([K_ROWS, HALF], f32r, tag="lhs")
            rhs_sb = cpool.tile([K_ROWS, N], f32r, tag="rhs")
            acc = cpool.tile([128, N], f16, tag="acc")
            rowmins = cpool.tile([128, STRIPS], f32, tag="rowmins")
            # row-path fold tree scratch (fp16 keeps DVE in 2x_1p mode)
            f1 = qpool.tile([128, 4096], f16, tag="f1")
            f2 = qpool.tile([128, 2048], f16, tag="f2")
            f3 = qpool.tile([128, 1024], f16, tag="f3")
            f4 = qpool.tile([128, 512], f16, tag="f4")
            # per-strip fold5 outputs land in 8-strip wave slots; one
            # segmented reduce per wave replaces 8 (fold6 + reduce) pairs
            f5buf = qpool.tile([128, 8, 256], f16, tag="f5buf")

            nc.sync.dma_start(lhs_sb[:, :], lhs)
            nc.sync.dma_start(rhs_sb[:, :], rhs)

            P = ppool.tile([128, 8 * 512], f32, tag="P")

            loop_ctx = (tc.For_i(0, reps, 1) if reps > 1
                        else contextlib.nullcontext())
            with loop_ctx:
                dst_prev = None
                for s in range(STRIPS):
                    dst = spool.tile([128, N], f16, tag="strip")
                    for g in range(4):
                        base = 2048 * (g % 2)
                        for j in range(4):
                            m = 4 * g + j
                            nc.tensor.matmul(
                                P[:, base + 512 * j: base + 512 * (j + 1)],
                                lhsT=lhs_sb[:, 128 * s: 128 * (s + 1)],
                                rhs=rhs_sb[:, 512 * m: 512 * (m + 1)],
                                start=True, stop=True,
                            )
                        nc.scalar.copy(dst[:, 2048 * g: 2048 * (g + 1)],
                                       P[:, base: base + 2048])
                    # row path: binary fold tree at 2x, then one 1x reduce
                    nc.vector.tensor_tensor(
                        out=f1[:, :], in0=dst[:, 0:4096], in1=dst[:, 4096:N],
                        op=AOp.min)
                    nc.vector.tensor_tensor(
                        out=f2[:, :], in0=f1[:, 0:2048], in1=f1[:, 2048:4096],
                        op=AOp.min)
                    nc.vector.tensor_tensor(
                        out=f3[:, :], in0=f2[:, 0:1024], in1=f2[:, 1024:2048],
                        op=AOp.min)
                    nc.vector.tensor_tensor(
                        out=f4[:, :], in0=f3[:, 0:512], in1=f3[:, 512:1024],
                        op=AOp.min)
                    nc.vector.tensor_tensor(
                        out=f5buf[:, s % 8, :], in0=f4[:, 0:256],
                        in1=f4[:, 256:512], op=AOp.min)
                    if s % 8 == 7:
                        nc.vector.tensor_reduce(
                            rowmins[:, s - 7:s + 1], f5buf[:, :, :],
                            axis=mybir.AxisListType.X, op=AOp.min)
                    # col path: acc initialized at s=1 from the first two
                    # strips (not at s=0) so the previous iteration's acc
                    # DMA-out overlaps ~12us of fresh compute before the
                    # first write-after-read on acc.
                    if s == 1:
                        nc.vector.tensor_tensor(
                            out=acc[:, :], in0=dst_prev[:, :], in1=dst[:, :],
                            op=AOp.min,
                        )
                    elif s > 1:
                        nc.vector.tensor_tensor(
                            out=acc[:, :], in0=acc[:, :], in1=dst[:, :],
                            op=AOp.min,
                        )
                    dst_prev = dst

                # split the 2MB result DMA across two DGE queues
                nc.sync.dma_start(out_acc[:, 0:N // 2], acc[:, 0:N // 2])
                nc.scalar.dma_start(out_acc[:, N // 2:N], acc[:, N // 2:N])
                nc.sync.dma_start(out_row, rowmins[:, :])

    nc.compile()
    return nc


def _rnd11(x):
    """Round-to-nearest keeping 11 explicit mantissa bits (the rounding the
    PE applies to float32r operands, measured on HW)."""
    xi = x.view(np.uint32).astype(np.uint64)
    out = ((xi + np.uint64(1 << 11)) & np.uint64(0xFFFFF000)).astype(np.uint32)
    return out.view(np.float32)


def _hilo(x):
    hi = _rnd11(np.ascontiguousarray(x, np.float32))
    lo = _rnd11((x - hi).astype(np.float32))
    return hi, lo


def _prep_core_inputs(template, source, c):
    b, h = divmod(c, 2)
    tch = template[b, h * HALF:(h + 1) * HALF]  # [4096, 3] rows
    sfull = source[b]  # [8192, 3] cols

    def sq(x):  # |x|^2 per point, fp32
        return (x * x).sum(axis=-1, dtype=np.float32)

    # Contraction layout (k: lhs row x rhs row):
    #   0-2 : -2*t_hi . s_hi     3-5 : -2*t_hi . s_lo
    #   6-8 : -2*t_lo . s_hi     9-11: -2*t_lo . s_lo
    #   12  : a2_hi * 1          13  : a2_lo * 1
    #   14  : 1 * b2_hi          15  : 1 * b2_lo
    v = (-2.0 * tch.T).astype(np.float32)  # [3, 4096]
    vh, vl = _hilo(v)
    a2h, a2l = _hilo(sq(tch)[None])
    ones_n = np.ones((1, tch.shape[0]), np.float32)
    lhs = np.concatenate([vh, vh, vl, vl, a2h, a2l, ones_n, ones_n], axis=0)

    w = np.ascontiguousarray(sfull.T, np.float32)  # [3, 8192]
    wh, wl = _hilo(w)
    b2h, b2l = _hilo(sq(sfull)[None])
    ones_m = np.ones((1, N), np.float32)
    rhs = np.concatenate([wh, wl, wh, wl, ones_m, ones_m, b2h, b2l], axis=0)

    return {
        "lhs": np.ascontiguousarray(lhs),
        "rhs": np.ascontiguousarray(rhs),
    }


def _run(template, source, trace=False):
    from concourse.bass_utils import run_bass_kernel_spmd

    template = np.asarray(template, np.float32)
    source = np.asarray(source, np.float32)
    assert template.shape == (B, N, 3) and source.shape == (B, N, 3)

    if "nc" not in _cache:
        _cache["nc"] = _build_bass()
    nc = _cache["nc"]

    in_maps = [_prep_core_inputs(template, source, c) for c in range(N_CORES)]
    res = run_bass_kernel_spmd(nc, in_maps, core_ids=list(range(N_CORES)),
                               trace=trace)

    acc = np.stack([np.asarray(r["out_acc"], np.float32)
                    for r in res.results])  # [8, 128, 8192]
    rm = np.stack([np.asarray(r["out_row"], np.float32)
                   for r in res.results])  # [8, 128, 32]

    # template -> source: rowmins are complete minima for this core's rows
    rowsqrt = np.sqrt(np.maximum(rm, 0.0)).sum(axis=(1, 2))  # [8]
    cost01 = (rowsqrt[0::2] + rowsqrt[1::2]) / N  # [4]

    # source -> template: min over partitions, then across the core pair
    colmin_core = acc.min(axis=1)  # [8, 8192]
    colmin = np.minimum(colmin_core[0::2], colmin_core[1::2])  # [4, 8192]
    cost10 = np.sqrt(np.maximum(colmin, 0.0)).mean(axis=1)  # [4]

    chamfer = ((cost01 + cost10) / 2.0).mean()
    return np.asarray(chamfer, dtype=np.float32), res


def kernel(template, source):
    val, _ = _run(template, source, trace=False)
    return val
